# revision 1
# baseline (speedup 1.0000x reference)
"""Trainium2 Bass kernel for nn_AttnMap: out = relu(einsum(dec,enc) @ W + bias).

Math: scores[b,t,hw,(q,g)] = sum_c dec[b,g,q,t,c] * enc[b,t,hw,(g,c)]
      out = relu(scores @ W + bias)
Fusion: out[b,t] = relu(enc[b,t] @ M_t + bias) with
      M_t[(g,c), f] = sum_q dec[b,q,t,(g,c)] * W[q*8+g, f]   ([256,256] per t)

Sharding: data-parallel over batch b across the 8 NeuronCores.

Per-core pipeline (t = 0..15):
  1. DMA enc_t [1024,256] -> SBUF [128, (chunk,C)] natural layout.
  2. M_t via 8 bf16 matmuls (K=16 over q, M=32, col-groups) -> PSUM -> f32r SBUF.
  3. enc_t transposed C-major via 16 exact fp32 PE transposes -> PSUM -> f32r SBUF.
  4. out chunks: 2 accumulating f32r matmuls (K=128 over C-half, N=256)
     (+ optional K=1 bf16 bias matmul) -> PSUM -> relu on ACT -> SBUF -> DMA out.
"""
import numpy as np
from contextlib import ExitStack

B, T, HW, C, F = 8, 16, 1024, 256, 256
G, CG, Q = 8, 32, 16  # heads, head dim, queries

_cache = {}


def _build(with_bias: bool, reps: int = 1, tune: dict | None = None):
    import concourse.tile as tile
    from concourse import bacc, mybir

    tune = dict(tune or {})
    BUFS_ENC = tune.get("bufs_enc", 4)
    BUFS_ENCT = tune.get("bufs_encT", 3)
    BUFS_OUT = tune.get("bufs_out", 4)
    DMA_T = tune.get("dma_t", 1)       # t's per enc/out DMA (1 or 2)
    ACT_M = tune.get("act_m", False)   # M evac on ACT instead of DVE
    MODE = tune.get("mode", "full")    # full|dma_only|no_stage2|no_transpose
    OUT_ON_ACT = tune.get("out_on_act", True)   # out DMA via ACT HWDGE ring
    ENC_BF16 = tune.get("enc_bf16", False)      # cast-DMA enc to bf16
    BF16 = tune.get("bf16", False)              # bf16 encT/M for stage-2
    CAST_TR = tune.get("cast_tr", False)        # pre-cast enc to bf16; bf16 transposes
    M_UPFRONT = tune.get("m_upfront", False)    # all M_t in one burst pre-loop
    TR_F32R = tune.get("tr_f32r", False)        # f32r transposes (1.5 c/r)
    SWDGE_IN = tune.get("swdge_in", False)      # odd-t enc loads via SWDGE
    BUFS_PT = tune.get("bufs_pt", 2)
    BUFS_PO = tune.get("bufs_po", 2)

    f32 = mybir.dt.float32
    f32r = mybir.dt.float32r
    bf16 = mybir.dt.bfloat16

    nc = bacc.Bacc("TRN2", target_bir_lowering=False, debug=False,
                   num_devices=8)

    enc_dram_dt = f32r if TR_F32R else f32
    t_enc = nc.dram_tensor("enc", [T, HW, C], enc_dram_dt,
                           kind="ExternalInput").ap()
    t_dec = nc.dram_tensor("dec", [Q * T, G * CG], f32,
                           kind="ExternalInput").ap()
    t_wp = nc.dram_tensor("wp", [Q, G * F], f32, kind="ExternalInput").ap()
    t_bias = nc.dram_tensor("bias", [1, F], f32, kind="ExternalInput").ap()
    t_id = nc.dram_tensor("ident", [128, 128], f32, kind="ExternalInput").ap()
    t_out = nc.dram_tensor("out", [T, HW, C], f32, kind="ExternalOutput").ap()

    with tile.TileContext(nc) as tc, ExitStack() as ctx:
        const = ctx.enter_context(tc.tile_pool(name="const", bufs=1))
        encp = ctx.enter_context(tc.tile_pool(name="encp", bufs=BUFS_ENC))
        encTp = ctx.enter_context(tc.tile_pool(name="encTp", bufs=BUFS_ENCT))
        outsp = ctx.enter_context(tc.tile_pool(name="outsp", bufs=BUFS_OUT))
        mp = ctx.enter_context(tc.tile_pool(name="mp", bufs=2))
        ps_t = ctx.enter_context(tc.tile_pool(name="ps_t", bufs=BUFS_PT,
                                              space="PSUM"))
        ps_m = ctx.enter_context(tc.tile_pool(name="ps_m", bufs=2,
                                              space="PSUM"))
        ps_o = ctx.enter_context(tc.tile_pool(name="ps_o", bufs=BUFS_PO,
                                              space="PSUM"))

        s_id = const.tile([128, 128], f32r if TR_F32R else f32, tag="ident")
        nc.sync.dma_start(s_id[:], t_id.bitcast(f32r) if TR_F32R else t_id)
        if CAST_TR:
            s_idb = const.tile([128, 128], bf16, tag="identb")
            nc.gpsimd.dma_start(s_idb[:], t_id)
        # dec as [q, (t, g, c)] bf16 (SWDGE cast-DMA)
        s_dq = const.tile([Q, T * C], bf16, tag="dq")
        nc.gpsimd.dma_start(s_dq[:], t_dec.rearrange("(q t) c -> q (t c)",
                                                     t=T))
        # W permuted+replicated on host to [q, (g, f)] bf16
        s_wp = const.tile([Q, G * F], bf16, tag="wp")
        nc.gpsimd.dma_start(s_wp[:], t_wp)
        if with_bias:
            s_ones = const.tile([1, 128], bf16, tag="ones")
            nc.gpsimd.memset(s_ones[:], 1.0)
            s_bias = const.tile([1, F], bf16, tag="bias")
            nc.gpsimd.dma_start(s_bias[:], t_bias)

        rep_loop = (tc.For_i(0, reps, 1,
                             hint_engines=(mybir.EngineType.PE,
                                           mybir.EngineType.DVE,
                                           mybir.EngineType.Activation,
                                           mybir.EngineType.SP))
                    if reps > 1 else None)
        if rep_loop is not None:
            ctx.enter_context(rep_loop)

        sdt = bf16 if BF16 else f32r
        mallp = ctx.enter_context(tc.tile_pool(name="mallp", bufs=1)) \
            if M_UPFRONT else None
        if M_UPFRONT:
            m_all = mallp.tile([128, T * 512], sdt, tag="mall")
            for tp in range(T // 2):
                pmu = ps_m.tile([128, 1024], f32, tag="pmu")
                for tl2 in range(2):
                    ti2 = tp * 2 + tl2
                    for gh in range(2):
                        for gm in range(4):
                            g = gh * 4 + gm
                            nc.tensor.matmul(
                                pmu[gm * 32:(gm + 1) * 32,
                                    tl2 * 512 + gh * 256:
                                    tl2 * 512 + (gh + 1) * 256],
                                s_dq[:, ti2 * C + g * CG:
                                     ti2 * C + (g + 1) * CG],
                                s_wp[:, g * F:(g + 1) * F],
                                tile_position=(0, gm * 32))
                nc.vector.tensor_copy(
                    m_all[:, tp * 1024:(tp + 1) * 1024], pmu[:])
        if MODE == "no_transpose":
            encT_fix = const.tile([128, 2048], sdt, tag="encT_fix")
            nc.gpsimd.memset(encT_fix[:], 0.5)
        if MODE == "dma_only_bf2":
            dummy_o = const.tile([128, 2048 * DMA_T], f32, tag="dummy_o")
            nc.gpsimd.memset(dummy_o[:], 0.25)

        out_eng = nc.scalar if OUT_ON_ACT else nc.sync
        enc_dt = bf16 if ENC_BF16 else (f32r if TR_F32R else f32)
        for tg in range(T // DMA_T):
            # ---- load enc: sbuf[p, tl*2048 + ch*256 + c] = enc[t, ch*128+p, c]
            enc_sb = encp.tile([128, 2048 * DMA_T], enc_dt, tag="enc")
            in_eng = (nc.gpsimd if (ENC_BF16 or (SWDGE_IN and tg % 2))
                      else nc.sync)
            in_eng.dma_start(
                enc_sb[:].rearrange("p (t ch c) -> p t ch c", t=DMA_T, ch=8),
                t_enc[tg * DMA_T:(tg + 1) * DMA_T].rearrange(
                    "t (ch p) c -> p t ch c", p=128))
            o_sb = outsp.tile([128, 2048 * DMA_T], f32, tag="o")

            if MODE == "dma_only":
                (nc.gpsimd if ENC_BF16 else out_eng).dma_start(
                    t_out[tg * DMA_T:(tg + 1) * DMA_T].rearrange(
                        "t (ch p) c -> p t ch c", p=128),
                    enc_sb[:].rearrange("p (t ch c) -> p t ch c",
                                        t=DMA_T, ch=8))
                continue
            if MODE == "dma_only_bf2":
                out_eng.dma_start(
                    t_out[tg * DMA_T:(tg + 1) * DMA_T].rearrange(
                        "t (ch p) c -> p t ch c", p=128),
                    dummy_o[:].rearrange("p (t ch c) -> p t ch c",
                                         t=DMA_T, ch=8))
                continue

            for tl in range(DMA_T):
                ti = tg * DMA_T + tl
                eb = tl * 2048   # enc_sb col base for this t
                ob = tl * 2048   # o_sb col base

                # ---- M_t: pm[gm*32+c, gh*256+f], bf16 matmuls K=16
                if M_UPFRONT:
                    m_sb = m_all[:, ti * 512:(ti + 1) * 512]
                pm = None if M_UPFRONT else ps_m.tile([128, 512], f32,
                                                      tag="pm")
                if not M_UPFRONT:
                    for gh in range(2):
                        for gm in range(4):
                            g = gh * 4 + gm
                            nc.tensor.matmul(
                                pm[gm * 32:(gm + 1) * 32,
                                   gh * 256:(gh + 1) * 256],
                                s_dq[:, ti * C + g * CG:
                                     ti * C + (g + 1) * CG],
                                s_wp[:, g * F:(g + 1) * F],
                                tile_position=(0, gm * 32))
                    m_sb = mp.tile([128, 512], sdt, tag="m")
                    if ACT_M:
                        nc.scalar.copy(m_sb[:], pm[:])
                    else:
                        nc.vector.tensor_copy(m_sb[:], pm[:])

                # ---- transpose enc_t -> encT[C%128, gh*1024 + ch*128 + hw']
                if CAST_TR:
                    enc_bf = encp.tile([128, 2048], bf16, tag="encbf")
                    for q4 in range(4):
                        cp_eng = nc.vector if q4 % 2 == 0 else nc.scalar
                        cp = (cp_eng.tensor_copy if q4 % 2 == 0
                              else cp_eng.copy)
                        cp(enc_bf[:, q4 * 512:(q4 + 1) * 512],
                           enc_sb[:, eb + q4 * 512: eb + (q4 + 1) * 512])
                    tr_src, tr_base, tr_id, tr_dt = enc_bf, 0, s_idb, bf16
                elif TR_F32R:
                    tr_src, tr_base, tr_id, tr_dt = \
                        enc_sb[:], eb, s_id[:], f32r
                else:
                    tr_src, tr_base, tr_id, tr_dt = enc_sb, eb, s_id, f32
                if MODE == "no_transpose":
                    encT = encT_fix
                else:
                    encT = encTp.tile([128, 2048], sdt, tag="encT")
                for pair in range(0 if MODE == "no_transpose" else 4):
                    if MODE == "transpose_only_nodve":
                        pt = ps_t.tile([128, 512], f32, tag="pt")
                        for i in range(2):
                            ch = pair * 2 + i
                            for gh in range(2):
                                nc.tensor.matmul(
                                    pt[:, i * 256 + gh * 128:
                                       i * 256 + (gh + 1) * 128],
                                    enc_sb[:, eb + ch * 256 + gh * 128:
                                           eb + ch * 256 + (gh + 1) * 128],
                                    s_id[:], is_transpose=True)
                        continue
                    pt = ps_t.tile([128, 512], tr_dt, tag="pt")
                    for i in range(2):
                        ch = pair * 2 + i
                        for gh in range(2):
                            nc.tensor.matmul(
                                pt[:, i * 256 + gh * 128:
                                   i * 256 + (gh + 1) * 128],
                                tr_src[:, tr_base + ch * 256 + gh * 128:
                                       tr_base + ch * 256 + (gh + 1) * 128],
                                tr_id[:], is_transpose=True)
                    pt_v = pt[:].rearrange("p (i gh x) -> p i gh x",
                                           i=2, gh=2)
                    encT_v = encT[:].rearrange("p (gh ch x) -> p ch gh x",
                                               gh=2, ch=8)
                    nc.vector.tensor_copy(
                        encT_v[:, pair * 2:(pair + 1) * 2, :, :], pt_v)

                if MODE in ("transpose_only", "transpose_only_nodve"):
                    continue
                if MODE == "no_stage2":
                    nc.sync.dma_start(
                        t_out[ti].rearrange("(ch p) c -> p ch c", p=128),
                        encT[:].bitcast(f32).rearrange("p (ch c) -> p ch c",
                                                       ch=16))
                    continue

                # ---- out chunks: po[hw', i*256+f] for ch = pair*2+i
                for pair in range(4):
                    po = ps_o.tile([128, 512], f32, tag="po")
                    for i in range(2):
                        ch = pair * 2 + i
                        for gh in range(2):
                            nc.tensor.matmul(
                                po[:, i * 256:(i + 1) * 256],
                                encT[:, gh * 1024 + ch * 128:
                                     gh * 1024 + (ch + 1) * 128],
                                m_sb[:, gh * 256:(gh + 1) * 256],
                                start=(gh == 0),
                                stop=(gh == 1 and not with_bias))
                        if with_bias:
                            nc.tensor.matmul(
                                po[:, i * 256:(i + 1) * 256],
                                s_ones[:], s_bias[:], start=False, stop=True,
                                skip_group_check=True)
                    nc.scalar.activation(
                        o_sb[:, ob + pair * 512: ob + (pair + 1) * 512],
                        po[:], mybir.ActivationFunctionType.Relu)

            if MODE not in ("no_stage2", "transpose_only",
                            "transpose_only_nodve"):
                out_eng.dma_start(
                    t_out[tg * DMA_T:(tg + 1) * DMA_T].rearrange(
                        "t (ch p) c -> p t ch c", p=128),
                    o_sb[:].rearrange("p (t ch c) -> p t ch c",
                                      t=DMA_T, ch=8))

    nc.compile()
    return nc


def _build_ilv(with_bias: bool, reps: int = 1, tune: dict | None = None):
    """Interleaved/software-pipelined build: stage-2 matmuls of t-1 are
    emitted between the transpose groups of t so real matmuls keep the PE
    HAM clock-gate warm (transpose-mode doesn't count as PE-busy)."""
    import concourse.tile as tile
    from concourse import bacc, mybir

    tune = dict(tune or {})
    BUFS_ENC = tune.get("bufs_enc", 2)
    BUFS_ENCT = tune.get("bufs_encT", 2)
    BUFS_OUT = tune.get("bufs_out", 2)
    BUFS_PT = tune.get("bufs_pt", 2)
    BUFS_PO = tune.get("bufs_po", 2)
    BF16 = tune.get("bf16", True)      # bf16 encT/M for stage-2
    MM_TR = tune.get("mm_tr", True)    # transposes as regular bf16 matmuls
    X2TR = tune.get("x2tr", False)     # emit transposes twice (probe)
    X2S2 = tune.get("x2s2", False)     # emit stage-2 groups twice (probe)
    FAT = tune.get("fat", False)       # 1024-col psum tiles, fewer sems

    f32 = mybir.dt.float32
    f32r = mybir.dt.float32r
    bf16 = mybir.dt.bfloat16
    Relu = mybir.ActivationFunctionType.Relu

    nc = bacc.Bacc("TRN2", target_bir_lowering=False, debug=False,
                   num_devices=8)

    t_enc = nc.dram_tensor("enc", [T, HW, C], f32, kind="ExternalInput").ap()
    t_dec = nc.dram_tensor("dec", [Q * T, G * CG], f32,
                           kind="ExternalInput").ap()
    t_wp = nc.dram_tensor("wp", [Q, G * F], f32, kind="ExternalInput").ap()
    t_bias = nc.dram_tensor("bias", [1, F], f32, kind="ExternalInput").ap()
    t_id = nc.dram_tensor("ident", [128, 128], f32, kind="ExternalInput").ap()
    t_out = nc.dram_tensor("out", [T, HW, C], f32, kind="ExternalOutput").ap()

    with tile.TileContext(nc) as tc, ExitStack() as ctx:
        const = ctx.enter_context(tc.tile_pool(name="const", bufs=1))
        encp = ctx.enter_context(tc.tile_pool(name="encp", bufs=BUFS_ENC))
        encTp = ctx.enter_context(tc.tile_pool(name="encTp", bufs=BUFS_ENCT))
        outsp = ctx.enter_context(tc.tile_pool(name="outsp", bufs=BUFS_OUT))
        mp = ctx.enter_context(tc.tile_pool(name="mp", bufs=2))
        ps_t = ctx.enter_context(tc.tile_pool(name="ps_t", bufs=BUFS_PT,
                                              space="PSUM"))
        ps_m = ctx.enter_context(tc.tile_pool(name="ps_m", bufs=2,
                                              space="PSUM"))
        ps_o = ctx.enter_context(tc.tile_pool(name="ps_o", bufs=BUFS_PO,
                                              space="PSUM"))

        s_id = const.tile([128, 128], f32, tag="ident")
        nc.sync.dma_start(s_id[:], t_id)
        if MM_TR:
            s_idb = const.tile([128, 128], bf16, tag="identb")
            nc.gpsimd.dma_start(s_idb[:], t_id)
        s_dq = const.tile([Q, T * C], bf16, tag="dq")
        nc.gpsimd.dma_start(s_dq[:], t_dec.rearrange("(q t) c -> q (t c)",
                                                     t=T))
        s_wp = const.tile([Q, G * F], bf16, tag="wp")
        nc.gpsimd.dma_start(s_wp[:], t_wp)
        if with_bias:
            s_ones = const.tile([1, 128], bf16, tag="ones")
            nc.gpsimd.memset(s_ones[:], 1.0)
            s_bias = const.tile([1, F], bf16, tag="bias")
            nc.gpsimd.dma_start(s_bias[:], t_bias)

        rep_loop = (tc.For_i(0, reps, 1,
                             hint_engines=(mybir.EngineType.PE,
                                           mybir.EngineType.DVE,
                                           mybir.EngineType.Activation,
                                           mybir.EngineType.SP))
                    if reps > 1 else None)
        if rep_loop is not None:
            ctx.enter_context(rep_loop)

        sdt = bf16 if BF16 else f32r

        def s_mms(prev, pair, po, pbase):
            """stage-2 matmuls for chunk-pair of a previous t into po."""
            ti_p, encT_p, m_p, o_p = prev
            for i in ([0, 1, 0, 1] if X2S2 else [0, 1]):
                ch = pair * 2 + i
                for gh in range(2):
                    nc.tensor.matmul(
                        po[:, pbase + i * 256: pbase + (i + 1) * 256],
                        encT_p[:, gh * 1024 + ch * 128:
                               gh * 1024 + (ch + 1) * 128],
                        m_p[:, gh * 256:(gh + 1) * 256],
                        start=(gh == 0),
                        stop=(gh == 1 and not with_bias))
                if with_bias:
                    nc.tensor.matmul(
                        po[:, pbase + i * 256: pbase + (i + 1) * 256],
                        s_ones[:], s_bias[:], start=False, stop=True,
                        skip_group_check=True)

        s_state = {}

        def s_group(prev, pair):
            o_p = prev[3]
            if FAT:
                if pair % 2 == 0:
                    s_state["po"] = ps_o.tile([128, 1024], f32, tag="po",
                                              name="pof")
                    s_mms(prev, pair, s_state["po"], 0)
                else:
                    po = s_state["po"]
                    s_mms(prev, pair, po, 512)
                    nc.scalar.activation(
                        o_p[:, (pair - 1) * 512:(pair + 1) * 512], po[:],
                        Relu)
            else:
                po = ps_o.tile([128, 512], f32, tag="po")
                s_mms(prev, pair, po, 0)
                nc.scalar.activation(
                    o_p[:, pair * 512:(pair + 1) * 512], po[:], Relu)

        prev = None
        for ti in range(T):
            enc_sb = encp.tile([128, 2048], f32, tag="enc")
            nc.sync.dma_start(
                enc_sb[:].rearrange("p (ch c) -> p ch c", ch=8),
                t_enc[ti].rearrange("(ch p) c -> p ch c", p=128))
            o_cur = outsp.tile([128, 2048], f32, tag="o")

            # M_t (bf16 K=16 col-group matmuls; real MMs -> HAM-warming)
            if FAT:
                pm = ps_t.tile([128, 512], f32, tag="pt", name="pm")
            else:
                pm = ps_m.tile([128, 512], f32, tag="pm")
            for gh in range(2):
                for gm in range(4):
                    g = gh * 4 + gm
                    nc.tensor.matmul(
                        pm[gm * 32:(gm + 1) * 32, gh * 256:(gh + 1) * 256],
                        s_dq[:, ti * C + g * CG: ti * C + (g + 1) * CG],
                        s_wp[:, g * F:(g + 1) * F],
                        tile_position=(0, gm * 32))
            m_cur = mp.tile([128, 512], sdt, tag="m")
            nc.vector.tensor_copy(m_cur[:], pm[:])

            if MM_TR:
                # cast enc to bf16 (DVE 2x-mode + ACT split); transposes as
                # REGULAR bf16 matmuls vs identity: fast + count as PE-busy
                enc_bf = encp.tile([128, 2048], bf16, tag="encbf")
                for q4 in range(4):
                    if q4 % 2 == 0:
                        nc.vector.tensor_copy(
                            enc_bf[:, q4 * 512:(q4 + 1) * 512],
                            enc_sb[:, q4 * 512:(q4 + 1) * 512])
                    else:
                        nc.scalar.copy(
                            enc_bf[:, q4 * 512:(q4 + 1) * 512],
                            enc_sb[:, q4 * 512:(q4 + 1) * 512])
                tr_src, tr_id, tr_kw = enc_bf, s_idb, {}
            else:
                tr_src, tr_id, tr_kw = enc_sb, s_id, {"is_transpose": True}
            encT_cur = encTp.tile([128, 2048], sdt, tag="encT")
            ptf = None
            for pair in range(4):
                if FAT:
                    if pair % 2 == 0:
                        ptf = ps_t.tile([128, 1024], f32, tag="pt")
                    pt = ptf[:, (pair % 2) * 512:(pair % 2 + 1) * 512]
                else:
                    pt0 = ps_t.tile([128, 512], f32, tag="pt")
                    pt = pt0[:]
                for rep2 in range(2 if X2TR else 1):
                    for i in range(2):
                        ch = pair * 2 + i
                        for gh in range(2):
                            nc.tensor.matmul(
                                pt[:, i * 256 + gh * 128:
                                   i * 256 + (gh + 1) * 128],
                                tr_src[:, ch * 256 + gh * 128:
                                       ch * 256 + (gh + 1) * 128],
                                tr_id[:], **tr_kw)
                encT_v = encT_cur[:].rearrange("p (gh ch x) -> p ch gh x",
                                               gh=2, ch=8)
                if FAT:
                    if pair % 2 == 1:
                        ptf_v = ptf[:].rearrange(
                            "p (pr i gh x) -> p (pr i) gh x", pr=2, i=2, gh=2)
                        nc.vector.tensor_copy(
                            encT_v[:, (pair - 1) * 2:(pair + 1) * 2, :, :],
                            ptf_v)
                else:
                    pt_v = pt.rearrange("p (i gh x) -> p i gh x", i=2, gh=2)
                    nc.vector.tensor_copy(
                        encT_v[:, pair * 2:(pair + 1) * 2, :, :], pt_v)
                if prev is not None:
                    s_group(prev, pair)   # keeps HAM warm between T groups

            if prev is not None:
                nc.scalar.dma_start(
                    t_out[prev[0]].rearrange("(ch p) c -> p ch c", p=128),
                    prev[3][:].rearrange("p (ch c) -> p ch c", ch=8))
            prev = (ti, encT_cur, m_cur, o_cur)

        for pair in range(4):
            s_group(prev, pair)
        nc.scalar.dma_start(
            t_out[prev[0]].rearrange("(ch p) c -> p ch c", p=128),
            prev[3][:].rearrange("p (ch c) -> p ch c", ch=8))

    nc.compile()
    return nc


def kernel(btn_dec, btn_enc, W, bias):
    from concourse.bass_utils import run_bass_kernel_spmd

    btn_dec = np.ascontiguousarray(np.asarray(btn_dec, dtype=np.float32))
    btn_enc = np.ascontiguousarray(np.asarray(btn_enc, dtype=np.float32))
    W = np.ascontiguousarray(np.asarray(W, dtype=np.float32))
    bias = np.ascontiguousarray(np.asarray(bias, dtype=np.float32))

    with_bias = bool(np.any(bias))
    key = ("nc", with_bias)
    if key not in _cache:
        _cache[key] = _build(with_bias)
    nc = _cache[key]

    # host layout prep (cheap reshapes only)
    wp = np.ascontiguousarray(
        W.reshape(Q, G, F).reshape(Q, G * F))  # W[q*8+g, f] -> [q, (g f)]
    ident = np.eye(128, dtype=np.float32)
    bias2 = bias.reshape(1, F)
    enc_r = btn_enc.reshape(B, T, HW, C)

    in_maps = [{"enc": enc_r[i], "dec": btn_dec[i], "wp": wp,
                "bias": bias2, "ident": ident} for i in range(B)]
    res = run_bass_kernel_spmd(nc, in_maps, core_ids=list(range(B)))
    out = np.stack([res.results[i]["out"] for i in range(B)])
    return out.reshape(B, T, 32, 32, C)



# revision 3
# speedup vs baseline: 1.3654x; 1.3654x over previous
"""Trainium2 Bass kernel for nn_AttnMap: out = relu(einsum(dec,enc) @ W + bias).

Math: scores[b,t,hw,(q,g)] = sum_c dec[b,g,q,t,c] * enc[b,t,hw,(g,c)]
      out = relu(scores @ W + bias)
Fusion: out[b,t] = relu(enc[b,t] @ M_t + bias) with
      M_t[(g,c), f] = sum_q dec[b,q,t,(g,c)] * W[q*8+g, f]   ([256,256] per t)

Sharding: data-parallel over batch b across the 8 NeuronCores.

This version is DMA-roofline focused:
  * enc is staged to HBM as bf16, pre-split [gh, t, hw, c'] so each
    (t-pair, gh) slab is a contiguous [2048, 128] block.
  * enc is loaded ALREADY TRANSPOSED via the DMA XBAR (dma transpose),
    so no PE transposes and no PSUM->SBUF evacuation of encT at all.
  * the output is written back as bf16 and upcast on the host.
This halves both DMA directions vs the f32 baseline and leaves PE with
only the real matmuls (M_t build + fused stage-2).

Per-core pipeline (t-pair groups, TP=2):
  1. 2 transposing DMAs (SP): encT[c', (gh, tl, hw)] <- enc[gh, tp] bf16.
  2. per t: M_t via 8 bf16 matmuls (K=16 over q) -> PSUM -> bf16 SBUF (DVE).
  3. per t: 4 accumulating bf16 matmul groups (K=128 x2 over C-halves,
     N=512) -> PSUM -> relu on ACT -> bf16 SBUF.
  4. 1 output DMA (ACT) per t-pair.
"""
import numpy as np
from contextlib import ExitStack

B, T, HW, C, F = 8, 16, 1024, 256, 256
G, CG, Q = 8, 32, 16  # heads, head dim, queries
TP = 2                # t's per DMA group

_cache = {}


def _build(with_bias: bool, reps: int = 1, tune: dict | None = None):
    import concourse.tile as tile
    from concourse import bacc, mybir

    tune = dict(tune or {})
    BUFS_ENCT = tune.get("bufs_encT", 3)
    BUFS_OUT = tune.get("bufs_out", 3)
    BUFS_PO = tune.get("bufs_po", 3)
    BUFS_PM = tune.get("bufs_pm", 2)
    MODE = tune.get("mode", "full")   # full | dma_only
    RELU_SPLIT = tune.get("relu_split", 0)  # po tiles per t evac'd on DVE

    f32 = mybir.dt.float32
    bf16 = mybir.dt.bfloat16
    Relu = mybir.ActivationFunctionType.Relu

    nc = bacc.Bacc("TRN2", target_bir_lowering=False, debug=False,
                   num_devices=8)

    # enc pre-split on host: [gh, t, hw, c'] bf16 (c = gh*128 + c')
    t_enc = nc.dram_tensor("enc", [2, T, HW, 128], bf16,
                           kind="ExternalInput").ap()
    # dec as [q, (t, g, c)] bf16 (host-permuted)
    t_dec = nc.dram_tensor("dec", [Q, T * C], bf16, kind="ExternalInput").ap()
    # W permuted on host to [q, (g, f)] bf16
    t_wp = nc.dram_tensor("wp", [Q, G * F], bf16, kind="ExternalInput").ap()
    if with_bias:
        t_bias = nc.dram_tensor("bias", [1, F], bf16,
                                kind="ExternalInput").ap()
    t_out = nc.dram_tensor("out", [T, HW, C], bf16, kind="ExternalOutput").ap()

    with tile.TileContext(nc) as tc, ExitStack() as ctx:
        const = ctx.enter_context(tc.tile_pool(name="const", bufs=1))
        encTp = ctx.enter_context(tc.tile_pool(name="encTp", bufs=BUFS_ENCT))
        outsp = ctx.enter_context(tc.tile_pool(name="outsp", bufs=BUFS_OUT))
        mp = ctx.enter_context(tc.tile_pool(name="mp", bufs=3))
        ps_m = ctx.enter_context(tc.tile_pool(name="ps_m", bufs=BUFS_PM,
                                              space="PSUM"))
        ps_o = ctx.enter_context(tc.tile_pool(name="ps_o", bufs=BUFS_PO,
                                              space="PSUM"))

        s_dq = const.tile([Q, T * C], bf16, tag="dq")
        nc.sync.dma_start(s_dq[:], t_dec)
        s_wp = const.tile([Q, G * F], bf16, tag="wp")
        nc.sync.dma_start(s_wp[:], t_wp)
        if with_bias:
            s_ones = const.tile([1, 128], bf16, tag="ones")
            nc.gpsimd.memset(s_ones[:], 1.0)
            s_bias = const.tile([1, F], bf16, tag="bias")
            nc.gpsimd.dma_start(s_bias[:], t_bias)

        rep_loop = (tc.For_i(0, reps, 1,
                             hint_engines=(mybir.EngineType.PE,
                                           mybir.EngineType.DVE,
                                           mybir.EngineType.Activation,
                                           mybir.EngineType.SP))
                    if reps > 1 else None)
        if rep_loop is not None:
            ctx.enter_context(rep_loop)

        for tp in range(T // TP):
            # transposing loads: encT[c', (gh, tl, hw)] <- enc[gh, tp-slab]
            encT = encTp.tile([128, 2 * TP * 1024], bf16, tag="encT")
            for gh in range(2):
                nc.sync.dma_start(
                    encT[:, gh * TP * 1024:(gh + 1) * TP * 1024],
                    t_enc[gh, tp * TP:(tp + 1) * TP].rearrange(
                        "t hw c -> (t hw) c"),
                    transpose=True)
            o_sb = outsp.tile([128, TP * 2048], bf16, tag="o")

            if MODE == "dma_only":
                nc.vector.memset(o_sb[:], 0.25)
                nc.scalar.dma_start(
                    t_out[tp * TP:(tp + 1) * TP].rearrange(
                        "t (ch p) c -> p t ch c", p=128),
                    o_sb[:].rearrange("p (t ch c) -> p t ch c", t=TP, ch=8))
                continue

            for tl in range(TP):
                ti = tp * TP + tl

                # ---- M_t: pm[gm*32+c, gh*256+f], bf16 matmuls K=16
                pm = ps_m.tile([128, 512], f32, tag="pm")
                for gh in range(2):
                    for gm in range(4):
                        g = gh * 4 + gm
                        nc.tensor.matmul(
                            pm[gm * 32:(gm + 1) * 32,
                               gh * 256:(gh + 1) * 256],
                            s_dq[:, ti * C + g * CG: ti * C + (g + 1) * CG],
                            s_wp[:, g * F:(g + 1) * F],
                            tile_position=(0, gm * 32))
                m_sb = mp.tile([128, 512], bf16, tag="m")
                nc.vector.tensor_copy(m_sb[:], pm[:])

                # ---- out chunks: po[hw', (i f)] for ch = pair*2+i
                for pair in range(4):
                    po = ps_o.tile([128, 512], f32, tag="po")
                    for i in range(2):
                        ch = pair * 2 + i
                        for gh in range(2):
                            nc.tensor.matmul(
                                po[:, i * 256:(i + 1) * 256],
                                encT[:, gh * TP * 1024 + tl * 1024
                                     + ch * 128:
                                     gh * TP * 1024 + tl * 1024
                                     + (ch + 1) * 128],
                                m_sb[:, gh * 256:(gh + 1) * 256],
                                start=(gh == 0),
                                stop=(gh == 1 and not with_bias))
                        if with_bias:
                            nc.tensor.matmul(
                                po[:, i * 256:(i + 1) * 256],
                                s_ones[:], s_bias[:], start=False, stop=True,
                                skip_group_check=True)
                    dst = o_sb[:, tl * 2048 + pair * 512:
                               tl * 2048 + (pair + 1) * 512]
                    if pair < RELU_SPLIT:
                        nc.vector.tensor_scalar_max(dst, po[:], 0.0)
                    else:
                        nc.scalar.activation(dst, po[:], Relu)

            nc.scalar.dma_start(
                t_out[tp * TP:(tp + 1) * TP].rearrange(
                    "t (ch p) c -> p t ch c", p=128),
                o_sb[:].rearrange("p (t ch c) -> p t ch c", t=TP, ch=8))

    nc.compile()
    return nc


def _host_prep(btn_dec, btn_enc, W, bias):
    """Full-batch device-input arrays (host-side layout prep + bf16 cast)."""
    import ml_dtypes
    bf16 = ml_dtypes.bfloat16

    enc = (btn_enc.reshape(B, T, HW, 2, 128)
           .transpose(0, 3, 1, 2, 4))            # [B, gh, T, HW, 128]
    enc_bf = np.ascontiguousarray(enc.astype(bf16))
    dec_bf = np.ascontiguousarray(
        btn_dec.reshape(B, Q, T * C).astype(bf16))  # [B, q, (t g c)]
    wp_bf = np.ascontiguousarray(
        W.reshape(Q, G * F).astype(bf16))           # [q, (g f)]
    d = {
        "enc": enc_bf,
        "dec": dec_bf,
        "wp": np.broadcast_to(wp_bf, (B,) + wp_bf.shape),
    }
    if np.any(bias):
        d["bias"] = np.broadcast_to(
            bias.reshape(1, F).astype(bf16), (B, 1, F))
    return d


def kernel(btn_dec, btn_enc, W, bias):
    from concourse.bass_utils import run_bass_kernel_spmd

    btn_dec = np.ascontiguousarray(np.asarray(btn_dec, dtype=np.float32))
    btn_enc = np.ascontiguousarray(np.asarray(btn_enc, dtype=np.float32))
    W = np.ascontiguousarray(np.asarray(W, dtype=np.float32))
    bias = np.ascontiguousarray(np.asarray(bias, dtype=np.float32))

    with_bias = bool(np.any(bias))
    key = ("nc", with_bias)
    if key not in _cache:
        _cache[key] = _build(with_bias)
    nc = _cache[key]

    ins = _host_prep(btn_dec, btn_enc, W, bias)
    in_maps = [{k: v[i] for k, v in ins.items()} for i in range(B)]
    res = run_bass_kernel_spmd(nc, in_maps, core_ids=list(range(B)))
    out = np.stack([np.asarray(res.results[i]["out"]) for i in range(B)])
    return out.astype(np.float32).reshape(B, T, 32, 32, C)


# revision 12
# speedup vs baseline: 1.7660x; 1.2934x over previous
"""Trainium2 Bass kernel for nn_AttnMap: out = relu(einsum(dec,enc) @ W + bias).

Math: scores[b,t,hw,(q,g)] = sum_c dec[b,g,q,t,c] * enc[b,t,hw,(g,c)]
      out = relu(scores @ W + bias)
Fusion: out[b,t] = relu(enc[b,t] @ M_t + bias) with
      M_t[(g,c), f] = sum_q dec[b,q,t,(g,c)] * W[q*8+g, f]   ([256,256] per t)
M_t is tiny (dec/W are <1% of the I/O) and is precomputed on the host;
the device does the two heavy parts: transposing enc (c must land on
partitions for the PE contraction) and the fused stage-2 matmul.

Sharding: data-parallel over batch b across the 8 NeuronCores.

DMA strategy (measured on this part): bf16 both directions; plain
descriptor DMAs in both directions overlap near-perfectly (~474 GB/s
mixed per core) while XBAR-transposing loads serialize badly against
concurrent writes — so the transpose is done on the PE as cheap bf16
identity matmuls instead. hw-index relabeling (hw = p*8 + x) makes both
the load and store APs 4KB-contiguous per partition.

Per-core pipeline (t-pair groups, TP=2):
  1. enc loads (SP HWDGE, 4KB descs): enc_sb[p, (t,x,c)] bf16.
  2. M loads (SP): m_sb[c', (t,gh,f)] bf16.
  3. per t: 16 transpose matmuls (PE, bf16 vs identity) -> PSUM (bf16)
     -> encT[c', (gh,x,p)] SBUF via DVE (gh=0) / Pool (gh=1) copies.
  4. per t: 4 accumulating bf16 matmul groups (K=128 x2 over C-halves)
     -> PSUM f32 -> relu (ACT x3 + DVE x1) -> o_sb bf16.
  5. out store (ACT HWDGE, 4KB descs) per t-pair.
"""
import numpy as np
from contextlib import ExitStack

B, T, HW, C, F = 8, 16, 1024, 256, 256
G, CG, Q = 8, 32, 16  # heads, head dim, queries
TP = 2                # t's per DMA group

_cache = {}


def _build(with_bias: bool, reps: int = 1, tune: dict | None = None):
    import concourse.tile as tile
    from concourse import bacc, mybir

    tune = dict(tune or {})
    BUFS_ENC = tune.get("bufs_enc", 3)
    BUFS_ENCT = tune.get("bufs_encT", 3)
    BUFS_OUT = tune.get("bufs_out", 3)
    BUFS_M = tune.get("bufs_m", 3)
    BUFS_PT = tune.get("bufs_pt", 2)   # per-gh transpose PSUM tiles
    BUFS_PO = tune.get("bufs_po", 3)
    PT_BF16 = tune.get("pt_bf16", True)
    RELU_DVE = tune.get("relu_dve", 1)   # po tiles per t relu'd on DVE
    EVAC_POOL = tune.get("evac_pool", False)  # gh=1 encT evac on Pool (PSUM
    # is not GPSIMD-accessible on TRN2, so this must stay False)
    MODE = tune.get("mode", "full")   # full | dma_only | dma_in | dma_out

    f32 = mybir.dt.float32
    bf16 = mybir.dt.bfloat16
    Relu = mybir.ActivationFunctionType.Relu

    nc = bacc.Bacc("TRN2", target_bir_lowering=False, debug=False,
                   num_devices=8)

    t_enc = nc.dram_tensor("enc", [T, HW, C], bf16,
                           kind="ExternalInput").ap()
    # host-precomputed M: [t, gh, c', f] bf16
    t_m = nc.dram_tensor("m", [T, 2, 128, F], bf16,
                         kind="ExternalInput").ap()
    t_id = nc.dram_tensor("identb", [128, 128], bf16,
                          kind="ExternalInput").ap()
    if with_bias:
        t_bias = nc.dram_tensor("bias", [1, F], bf16,
                                kind="ExternalInput").ap()
    t_out = nc.dram_tensor("out", [T, HW, C], bf16, kind="ExternalOutput").ap()

    with tile.TileContext(nc) as tc, ExitStack() as ctx:
        const = ctx.enter_context(tc.tile_pool(name="const", bufs=1))
        encp = ctx.enter_context(tc.tile_pool(name="encp", bufs=BUFS_ENC))
        encTp = ctx.enter_context(tc.tile_pool(name="encTp", bufs=BUFS_ENCT))
        outsp = ctx.enter_context(tc.tile_pool(name="outsp", bufs=BUFS_OUT))
        mp = ctx.enter_context(tc.tile_pool(name="mp", bufs=BUFS_M))
        ps_t0 = ctx.enter_context(tc.tile_pool(name="ps_t0", bufs=BUFS_PT,
                                               space="PSUM"))
        ps_t1 = ctx.enter_context(tc.tile_pool(name="ps_t1", bufs=BUFS_PT,
                                               space="PSUM"))
        ps_o = ctx.enter_context(tc.tile_pool(name="ps_o", bufs=BUFS_PO,
                                              space="PSUM"))

        s_id = const.tile([128, 128], bf16, tag="identb")
        nc.sync.dma_start(s_id[:], t_id)
        if with_bias:
            s_ones = const.tile([1, 128], bf16, tag="ones")
            nc.gpsimd.memset(s_ones[:], 1.0)
            s_bias = const.tile([1, F], bf16, tag="bias")
            nc.gpsimd.dma_start(s_bias[:], t_bias)

        dumo = None
        if MODE.startswith("dma"):
            dumo = const.tile([128, TP * 2048], bf16, tag="dumo")
            nc.vector.memset(dumo[:], 0.25)

        rep_loop = (tc.For_i(0, reps, 1,
                             hint_engines=(mybir.EngineType.PE,
                                           mybir.EngineType.DVE,
                                           mybir.EngineType.Activation,
                                           mybir.EngineType.SP,
                                           mybir.EngineType.Pool))
                    if reps > 1 else None)
        if rep_loop is not None:
            ctx.enter_context(rep_loop)

        ptdt = bf16 if PT_BF16 else f32

        for tp in range(T // TP):
            # ---- loads: enc_sb[p, (t, x, c)] = enc[t, p*8+x, c]
            enc_sb = encp.tile([128, TP * 2048], bf16, tag="enc")
            if MODE != "dma_out":
                nc.sync.dma_start(
                    enc_sb[:].rearrange("p (t xc) -> p t xc", t=TP),
                    t_enc[tp * TP:(tp + 1) * TP].rearrange(
                        "t (p x) c -> p t (x c)", p=128))
            o_sb = outsp.tile([128, TP * 2048], bf16, tag="o")

            if MODE.startswith("dma"):
                do_out = MODE in ("dma_only", "dma_out") or tp == 0
                if do_out:
                    nc.scalar.dma_start(
                        t_out[tp * TP:(tp + 1) * TP].rearrange(
                            "t (p x) c -> p t (x c)", p=128),
                        dumo[:].rearrange("p (t xc) -> p t xc", t=TP))
                continue

            # m_sb[c', (t, gh, f)]
            m_sb = mp.tile([128, TP * 512], bf16, tag="m")
            nc.sync.dma_start(
                m_sb[:].rearrange("c (t gh f) -> c t gh f", t=TP, gh=2),
                t_m[tp * TP:(tp + 1) * TP].rearrange(
                    "t gh c f -> c t gh f"))

            for tl in range(TP):
                # ---- transposes: pt_gh[c', (x, p)] = enc[t, p*8+x, ghc']
                pts = []
                for gh in range(2):
                    pool = ps_t0 if gh == 0 else ps_t1
                    ptg = pool.tile([128, 1024], ptdt, tag=f"pt{gh}")
                    for x in range(8):
                        nc.tensor.matmul(
                            ptg[:, x * 128:(x + 1) * 128],
                            enc_sb[:, tl * 2048 + x * 256 + gh * 128:
                                   tl * 2048 + x * 256 + (gh + 1) * 128],
                            s_id[:], is_transpose=PT_BF16)
                    pts.append(ptg)
                encT = encTp.tile([128, 2048], bf16, tag="encT")
                nc.vector.tensor_copy(encT[:, 0:1024], pts[0][:])
                if EVAC_POOL:
                    nc.gpsimd.tensor_copy(encT[:, 1024:2048], pts[1][:])
                else:
                    nc.vector.tensor_copy(encT[:, 1024:2048], pts[1][:])

                # ---- stage-2: po[p, (i f)] for x = pair*2+i
                for pair in range(4):
                    po = ps_o.tile([128, 512], f32, tag="po")
                    for i in range(2):
                        x = pair * 2 + i
                        for gh in range(2):
                            nc.tensor.matmul(
                                po[:, i * 256:(i + 1) * 256],
                                encT[:, gh * 1024 + x * 128:
                                     gh * 1024 + (x + 1) * 128],
                                m_sb[:, (tl * 2 + gh) * 256:
                                     (tl * 2 + gh + 1) * 256],
                                start=(gh == 0),
                                stop=(gh == 1 and not with_bias))
                        if with_bias:
                            nc.tensor.matmul(
                                po[:, i * 256:(i + 1) * 256],
                                s_ones[:], s_bias[:], start=False, stop=True,
                                skip_group_check=True)
                    dst = o_sb[:, tl * 2048 + pair * 512:
                               tl * 2048 + (pair + 1) * 512]
                    if pair < RELU_DVE:
                        nc.vector.tensor_scalar_max(dst, po[:], 0.0)
                    else:
                        nc.scalar.activation(dst, po[:], Relu)

            nc.scalar.dma_start(
                t_out[tp * TP:(tp + 1) * TP].rearrange(
                    "t (p x) c -> p t (x c)", p=128),
                o_sb[:].rearrange("p (t xc) -> p t xc", t=TP))

    nc.compile()
    return nc


def _host_prep(btn_dec, btn_enc, W, bias):
    """Full-batch device-input arrays (host-side layout prep + bf16 cast)."""
    import ml_dtypes
    bf16 = ml_dtypes.bfloat16

    enc_bf = np.ascontiguousarray(
        btn_enc.reshape(B, T, HW, C).astype(bf16))   # [B, T, HW, C]
    # M[b,t,g,c,f] = sum_q dec[b,q,t,g,c] * W[q,g,f]
    dec5 = btn_dec.reshape(B, Q, T, G, CG)
    W3 = W.reshape(Q, G, F)
    M = np.einsum("bqtgc,qgf->btgcf", dec5, W3, optimize=True)
    M = M.reshape(B, T, 2, 128, F).astype(bf16)      # [B, t, gh, c', f]
    ident = np.eye(128, dtype=np.float32).astype(bf16)
    d = {
        "enc": enc_bf,
        "m": np.ascontiguousarray(M),
        "identb": np.broadcast_to(ident, (B,) + ident.shape),
    }
    if np.any(bias):
        d["bias"] = np.broadcast_to(
            bias.reshape(1, F).astype(bf16), (B, 1, F))
    return d


def kernel(btn_dec, btn_enc, W, bias):
    from concourse.bass_utils import run_bass_kernel_spmd

    btn_dec = np.ascontiguousarray(np.asarray(btn_dec, dtype=np.float32))
    btn_enc = np.ascontiguousarray(np.asarray(btn_enc, dtype=np.float32))
    W = np.ascontiguousarray(np.asarray(W, dtype=np.float32))
    bias = np.ascontiguousarray(np.asarray(bias, dtype=np.float32))

    with_bias = bool(np.any(bias))
    key = ("nc", with_bias)
    if key not in _cache:
        _cache[key] = _build(with_bias)
    nc = _cache[key]

    ins = _host_prep(btn_dec, btn_enc, W, bias)
    in_maps = [{k: v[i] for k, v in ins.items()} for i in range(B)]
    res = run_bass_kernel_spmd(nc, in_maps, core_ids=list(range(B)))
    out = np.stack([np.asarray(res.results[i]["out"]) for i in range(B)])
    return out.astype(np.float32).reshape(B, T, 32, 32, C)
